# revision 13
# baseline (speedup 1.0000x reference)
"""Trainium2 Bass kernel for nn_CLM_23038204575917 (dense transformer CLM).

Sequence-parallel sharding: DP=2 over batch x SP=4 over tokens.
  core c (0..7): batch g = c//4, token quarter r = c%4
  (tokens [256r, 256r+256) of batch g). Full (unsharded) weights per
  core, streamed through SBUF in column strips. Per layer exactly ONE
  collective: an AllGather of locally-projected K and V' (packed in one
  ch-major buffer); attention out-projection and the FFN are token-local
  (no AllReduce at all). One final 8-way AllGather of the normalized
  hidden states feeds a vocab/8-sharded lm_head.

Activations transposed [E, tok] bf16; LN gamma/beta folded into weights
host-side; softmax without max-subtraction, causal mask multiplicative
after exp; softmax denominator via per-head ones-columns in V'.
"""

import contextlib
import ctypes
import sys
import types

import numpy as np

sys.path.insert(0, "/opt/trn_rl_repo")

import ml_dtypes

bf16 = ml_dtypes.bfloat16

# ---------------------------------------------------------------- ntff hook
# Allows run_bass_kernel_spmd(trace=True) / BASS_TRACE=1 to profile through
# the axon PJRT plugin even though the image's antenv lacks axon_hooks.
if "antenv.axon_hooks" not in sys.modules:
    def _ntff_profile_via_ctypes(so_path):
        try:
            lib = ctypes.CDLL(so_path)
        except OSError:
            return None
        if not hasattr(lib, "axon_start_nrt_profile"):
            return None
        lib.axon_start_nrt_profile.argtypes = [ctypes.POINTER(ctypes.c_int64), ctypes.c_size_t]
        lib.axon_start_nrt_profile.restype = ctypes.c_int64
        lib.axon_stop_nrt_profile.argtypes = [ctypes.c_char_p]
        lib.axon_stop_nrt_profile.restype = ctypes.c_int64

        @contextlib.contextmanager
        def _hook(output_dir, device_ids):
            import jax
            jax.devices()
            if device_ids:
                ids = (ctypes.c_int64 * len(device_ids))(*device_ids)
                rc = lib.axon_start_nrt_profile(ids, len(device_ids))
            else:
                rc = lib.axon_start_nrt_profile(None, 0)
            if rc != 0:
                raise RuntimeError(f"axon_start_nrt_profile rc={rc}")
            try:
                yield
            finally:
                n = lib.axon_stop_nrt_profile(str(output_dir).encode())
                print(f"ntff profile: {n} file(s) -> {output_dir}", file=sys.stderr)

        return _hook

    _mod = types.ModuleType("antenv.axon_hooks")
    _mod._hook = _ntff_profile_via_ctypes("/opt/axon/libaxon_pjrt.so")
    _mod.get_axon_ntff_profile_hook = lambda: _mod._hook
    _mod.set_axon_ntff_profile_hook = lambda h: setattr(_mod, "_hook", h)
    sys.modules["antenv.axon_hooks"] = _mod

import concourse.bass as bass
import concourse.tile as tile
from concourse import mybir
from concourse.bass_utils import run_bass_kernel_spmd

DT = mybir.dt
AF = mybir.ActivationFunctionType
ALU = mybir.AluOpType

# Model dims
V, T, E, H, L, FFD = 32000, 1024, 1024, 16, 4, 4096
HD = 64
NCORES = 8
SP = 4                   # sequence-parallel degree within a batch group
TOK = T // SP            # own tokens per core = 256
ET = E // 128            # 8 e-tiles
FT = FFD // 128          # 32 f-tiles
VROWS = H * (HD + 1)     # V' channel rows = 1040 (64 ch + 1 ones per head)
VRT = 9                  # V' row tiles (8 full + 1x16)
AGR = E + VROWS          # AG buffer rows per rank = 2064
VS = V // NCORES         # vocab slice per core = 4000
VSP = 4032               # padded to 8*504
VCH = 504                # lm_head psum column chunk
GROUPS4 = [[0, 1, 2, 3], [4, 5, 6, 7]]
GROUPS8 = [[0, 1, 2, 3, 4, 5, 6, 7]]


def _split_sync_waits(nc, max_waits=1):
    """This env's walrus rejects >1 sem-wait per instruction; move excess
    waits onto same-engine NoOps inserted just before."""
    for fn in nc.m.functions:
        for bb in fn.blocks:
            new_list = []
            for ins in bb.instructions:
                si = ins.sync_info
                if si is not None and si.on_wait and len(si.on_wait) > max_waits:
                    waits = list(si.on_wait)
                    extra, keep = waits[:-max_waits], waits[-max_waits:]
                    for k in range(0, len(extra), max_waits):
                        nop = mybir.InstNoOp(name=f"{ins.name}-ws{k}", ins=[], outs=[])
                        nop.engine = ins.engine
                        nop.sync_info = mybir.SyncInfo(
                            on_wait=extra[k:k + max_waits], on_update=[])
                        new_list.append(nop)
                    si.on_wait = keep
                new_list.append(ins)
            bb.instructions[:] = new_list


def _build_program():
    nc = bass.Bass()
    inp = {}

    def din(name, shape, dt=DT.bfloat16):
        inp[name] = nc.dram_tensor(name, list(shape), dt, kind="ExternalInput")
        return inp[name]

    x0T_d = din("x0T", (E, TOK))
    masks_d = din("masks", (8, 128, TOK))
    invE_d = din("invE", (128, 1))
    ones256_d = din("ones256", (1, TOK))
    ones64f_d = din("ones64f", (1, 64), DT.float32)
    embT_d = din("embT", (E, VSP))
    for l in range(L):
        din(f"wqs{l}", (ET, 128, E)); din(f"bq{l}", (128, ET), DT.float32)
        din(f"wks{l}", (ET, 128, E)); din(f"bk{l}", (128, ET), DT.float32)
        din(f"wvs{l}", (VRT, 128, E)); din(f"bvr{l}", (1, VRT * 128))
        din(f"wps{l}", (ET, 128, E))
        din(f"w1s{l}", (FT, 128, E)); din(f"bf{l}", (128, FT), DT.float32)
        din(f"w2s{l}", (ET, 128, FFD)); din(f"b2r{l}", (1, E))
    out_d = nc.dram_tensor("out", [2 * T, VSP], DT.float32, kind="ExternalOutput")

    with tile.TileContext(nc) as tc, contextlib.ExitStack() as ctx:
        cpool = ctx.enter_context(tc.tile_pool(name="const", bufs=1))
        xpool = ctx.enter_context(tc.tile_pool(name="x", bufs=1))
        hpool = ctx.enter_context(tc.tile_pool(name="h", bufs=1))
        qpool = ctx.enter_context(tc.tile_pool(name="q", bufs=1))
        stg = ctx.enter_context(tc.tile_pool(name="stg", bufs=3))
        wqp = ctx.enter_context(tc.tile_pool(name="wq", bufs=3))
        wkp = ctx.enter_context(tc.tile_pool(name="wk", bufs=3))
        wvp = ctx.enter_context(tc.tile_pool(name="wv", bufs=3))
        wpp = ctx.enter_context(tc.tile_pool(name="wp", bufs=3))
        w1p = ctx.enter_context(tc.tile_pool(name="w1", bufs=8))
        w2p = ctx.enter_context(tc.tile_pool(name="w2", bufs=2))
        biasp = ctx.enter_context(tc.tile_pool(name="bias", bufs=1))
        ktp = ctx.enter_context(tc.tile_pool(name="kt", bufs=1))
        vtp = ctx.enter_context(tc.tile_pool(name="vt", bufs=1))
        wexp = ctx.enter_context(tc.tile_pool(name="wex", bufs=1))
        o2p = ctx.enter_context(tc.tile_pool(name="o2", bufs=1))
        ffp = ctx.enter_context(tc.tile_pool(name="ff", bufs=1))
        rowp = ctx.enter_context(tc.tile_pool(name="rows", bufs=1))
        embp = ctx.enter_context(tc.tile_pool(name="emb", bufs=2))
        hfp = ctx.enter_context(tc.tile_pool(name="hf", bufs=1))
        lsbp = ctx.enter_context(tc.tile_pool(name="lsb", bufs=3))
        dram = ctx.enter_context(tc.tile_pool(name="dram", bufs=1, space="DRAM"))
        ps_mm = ctx.enter_context(tc.tile_pool(name="psmm", bufs=2, space="PSUM"))
        ps_w = ctx.enter_context(tc.tile_pool(name="psw", bufs=3, space="PSUM"))
        ps_o = ctx.enter_context(tc.tile_pool(name="pso", bufs=2, space="PSUM"))

        # ---- constants
        invE = cpool.tile([128, 1], DT.bfloat16, tag="invE", name="invE")
        nc.sync.dma_start(invE[:], invE_d[:])
        ones256 = cpool.tile([1, TOK], DT.bfloat16, tag="ones256", name="ones256")
        nc.sync.dma_start(ones256[:], ones256_d[:])
        ones64f = cpool.tile([1, 64], DT.float32, tag="ones64f", name="ones64f")
        nc.sync.dma_start(ones64f[:], ones64f_d[:])
        ones128f = cpool.tile([1, 128], DT.float32, tag="ones128f", name="ones128f")
        nc.gpsimd.memset(ones128f[:], 1.0)
        eps1 = cpool.tile([1, 1], DT.float32, tag="eps1", name="eps1")
        nc.gpsimd.memset(eps1[:], 1e-5)
        masks = [cpool.tile([128, TOK], DT.bfloat16, tag=f"mask{j}", name=f"mask{j}")
                 for j in range(8)]
        for j in range(8):
            nc.sync.dma_start(masks[j][:], masks_d[j])

        # ---- residual x[et] = x_T[e-tile, own tokens]
        x = [xpool.tile([128, TOK], DT.bfloat16, tag=f"x{et}", name=f"x{et}")
             for et in range(ET)]
        for et in range(ET):
            nc.sync.dma_start(x[et][:], x0T_d[et * 128:(et + 1) * 128, :])

        h = [hpool.tile([128, TOK], DT.bfloat16, tag=f"h{et}", name=f"h{et}")
             for et in range(ET)]

        def ln_own(tag):
            """h[et] = (x - mu) * rstd over own TOK tokens (bf16)."""
            mom = ps_w.tile([33, TOK], DT.float32, tag="w", name=f"mom{tag}")
            mu_ps, m2_ps = mom[0:1, :], mom[32:33, :]
            for et in range(ET):
                nc.tensor.matmul(mu_ps, invE[:], x[et][:],
                                 start=(et == 0), stop=(et == ET - 1))
            for et in range(ET):
                xsq = hpool.tile([128, TOK], DT.bfloat16, tag="xsq", bufs=3,
                                 name=f"xsq{tag}")
                nc.vector.tensor_tensor(xsq[:], x[et][:], x[et][:], op=ALU.mult)
                nc.tensor.matmul(m2_ps, invE[:], xsq[:],
                                 start=(et == 0), stop=(et == ET - 1))
            mu = rowp.tile([1, TOK], DT.float32, tag="mu_sb", name="mu_sb")
            nc.scalar.activation(mu[:], mu_ps, AF.Identity)
            var = rowp.tile([1, TOK], DT.float32, tag="var_sb", name="var_sb")
            mu2 = rowp.tile([1, TOK], DT.float32, tag="mu2_sb", name="mu2_sb")
            nc.vector.tensor_tensor(mu2[:], mu[:], mu[:], op=ALU.mult)
            nc.vector.tensor_tensor(var[:], m2_ps, mu2[:], op=ALU.subtract)
            sd = rowp.tile([1, TOK], DT.float32, tag="sd_sb", name="sd_sb")
            nc.scalar.activation(sd[:], var[:], AF.Sqrt, bias=eps1[:])
            a_row = rowp.tile([1, TOK], DT.float32, tag="a_sb", name="a_sb")
            nc.vector.reciprocal(a_row[:], sd[:])
            c_row = rowp.tile([1, TOK], DT.float32, tag="c_sb", name="c_sb")
            nc.vector.tensor_tensor(c_row[:], mu[:], a_row[:], op=ALU.mult)
            ab_ps = ps_w.tile([128, TOK], DT.float32, tag="w", name=f"ab{tag}")
            nc.tensor.matmul(ab_ps[:], ones128f[:], a_row[:], start=True, stop=True)
            cb_ps = ps_w.tile([128, TOK], DT.float32, tag="w", name=f"cb{tag}")
            nc.tensor.matmul(cb_ps[:], ones128f[:], c_row[:], start=True, stop=True)
            a_b = rowp.tile([128, TOK], DT.bfloat16, tag="ab_sb", bufs=2, name="ab_sb")
            nc.scalar.activation(a_b[:], ab_ps[:], AF.Identity)
            c_b = rowp.tile([128, TOK], DT.bfloat16, tag="cb_sb", bufs=2, name="cb_sb")
            nc.scalar.activation(c_b[:], cb_ps[:], AF.Identity)
            for et in range(ET):
                tmp = hpool.tile([128, TOK], DT.bfloat16, tag="lnt", bufs=2,
                                 name=f"lnt{tag}")
                nc.vector.tensor_tensor(tmp[:], x[et][:], a_b[:], op=ALU.mult)
                nc.vector.tensor_tensor(h[et][:], tmp[:], c_b[:], op=ALU.subtract)

        for l in range(L):
            # layer biases
            bqt = biasp.tile([128, ET], DT.float32, tag="bqt", name="bqt")
            nc.sync.dma_start(bqt[:], inp[f"bq{l}"][:])
            bkt = biasp.tile([128, ET], DT.float32, tag="bkt", name="bkt")
            nc.sync.dma_start(bkt[:], inp[f"bk{l}"][:])
            bvr = biasp.tile([1, VRT * 128], DT.bfloat16, tag="bvr", name="bvr")
            nc.sync.dma_start(bvr[:], inp[f"bvr{l}"][:])
            bft = biasp.tile([128, FT], DT.float32, tag="bft", name="bft")
            nc.sync.dma_start(bft[:], inp[f"bf{l}"][:])
            b2r = biasp.tile([1, E], DT.bfloat16, tag="b2r", name="b2r")
            nc.sync.dma_start(b2r[:], inp[f"b2r{l}"][:])

            # ---- LN1 -> h
            ln_own(f"l{l}a")

            agkin = dram.tile([E, TOK], DT.bfloat16, tag=f"agkin{l}", name="agkin")
            agkout = dram.tile([SP * E, TOK], DT.bfloat16, tag=f"agkout{l}",
                               name="agkout")
            agvin = dram.tile([VROWS, TOK], DT.bfloat16, tag=f"agvin{l}",
                              name="agvin")
            agvout = dram.tile([SP * VROWS, TOK], DT.bfloat16, tag=f"agvout{l}",
                               name="agvout")

            # ---- K projection (own tokens, all channels) -> AG-K in
            for ct in range(ET):
                wk_t = wkp.tile([128, E], DT.bfloat16, tag="wks", name=f"wk{l}_{ct}")
                nc.sync.dma_start(wk_t[:], inp[f"wks{l}"][ct])
                pk = ps_mm.tile([128, TOK], DT.float32, tag="mm", name="pk")
                for et in range(ET):
                    nc.tensor.matmul(pk[:], wk_t[:, et * 128:(et + 1) * 128],
                                     h[et][:], start=(et == 0), stop=(et == ET - 1))
                ksb = stg.tile([128, TOK], DT.bfloat16, tag="ksb", name="ksb")
                nc.scalar.activation(ksb[:], pk[:], AF.Identity,
                                     bias=bkt[:, ct:ct + 1])
                nc.gpsimd.dma_start(agkin[128 * ct:128 * (ct + 1), :], ksb[:])
            nc.gpsimd.collective_compute(
                "AllGather", ALU.bypass, replica_groups=GROUPS4,
                ins=[agkin.opt()], outs=[agkout.opt()])

            # ---- V' projection (ch-major, ones rows via bias matmul)
            for vr in range(VRT):
                rows = 128 if vr < VRT - 1 else VROWS - 128 * (VRT - 1)
                wv_t = wvp.tile([128, E], DT.bfloat16, tag="wvs", name=f"wv{l}_{vr}")
                nc.sync.dma_start(wv_t[:], inp[f"wvs{l}"][vr])
                pv = ps_mm.tile([128, TOK], DT.float32, tag="mm", name="pv")
                for et in range(ET):
                    nc.tensor.matmul(pv[:], wv_t[:, et * 128:(et + 1) * 128],
                                     h[et][:], start=(et == 0), stop=False)
                nc.tensor.matmul(pv[:], bvr[0:1, 128 * vr:128 * (vr + 1)],
                                 ones256[:], start=False, stop=True)
                vsb = stg.tile([128, TOK], DT.bfloat16, tag="vsb", name="vsb")
                nc.vector.tensor_copy(vsb[:], pv[:])
                nc.gpsimd.dma_start(
                    agvin[128 * vr:128 * vr + rows, :], vsb[0:rows, :])
            nc.gpsimd.collective_compute(
                "AllGather", ALU.bypass, replica_groups=GROUPS4,
                ins=[agvin.opt()], outs=[agvout.opt()])

            # ---- Q projection (overlaps the AllGather)
            qT = [qpool.tile([128, TOK], DT.bfloat16, tag=f"qT{ct}", name=f"qT{ct}")
                  for ct in range(ET)]
            for ct in range(ET):
                wq_t = wqp.tile([128, E], DT.bfloat16, tag="wqs", name=f"wq{l}_{ct}")
                nc.sync.dma_start(wq_t[:], inp[f"wqs{l}"][ct])
                pq = ps_mm.tile([128, TOK], DT.float32, tag="mm", name="pq")
                for et in range(ET):
                    nc.tensor.matmul(pq[:], wq_t[:, et * 128:(et + 1) * 128],
                                     h[et][:], start=(et == 0), stop=(et == ET - 1))
                nc.scalar.activation(qT[ct][:], pq[:], AF.Identity,
                                     bias=bqt[:, ct:ct + 1])

            # ---- gather K (ch-major) and V' (token-major via DMA transpose)
            kT = [ktp.tile([128, T], DT.bfloat16, tag=f"kT{ct}", name=f"kT{ct}")
                  for ct in range(ET)]
            for ct in range(ET):
                for b in range(SP):
                    nc.scalar.dma_start(
                        kT[ct][:, TOK * b:TOK * (b + 1)],
                        agkout[E * b + 128 * ct:E * b + 128 * (ct + 1), :])
            v = [vtp.tile([128, VROWS], DT.bfloat16, tag=f"v{kt}", name=f"v{kt}")
                 for kt in range(8)]
            for kt in range(8):
                b, hb = kt // 2, kt % 2
                nc.scalar.dma_start(
                    v[kt][:],
                    agvout[VROWS * b:VROWS * (b + 1),
                           128 * hb:128 * (hb + 1)],
                    transpose=True)

            # ---- attention: 4 waves of 4 heads; all 8 k-blocks, masks gate
            o2 = [o2p.tile([128, TOK], DT.bfloat16, tag=f"o2_{ct}", name=f"o2_{ct}")
                  for ct in range(ET)]
            for wv_i in range(8):
                heads = range(2 * wv_i, 2 * wv_i + 2)
                oacc = {hh: ps_o.tile([65, TOK], DT.float32, tag="o",
                                      name=f"oacc{hh}") for hh in heads}
                for kt in range(8):
                    for hh in heads:
                        ct, sub = hh // 2, hh % 2
                        pw = ps_w.tile([128, TOK], DT.float32, tag="w", name="pw")
                        nc.tensor.matmul(
                            pw[:],
                            kT[ct][64 * sub:64 * sub + 64,
                                   128 * kt:128 * (kt + 1)],
                            qT[ct][64 * sub:64 * sub + 64, :],
                            start=True, stop=True)
                        tmp = wexp.tile([128, TOK], DT.bfloat16, tag="wxt",
                                        bufs=3, name="wxt")
                        nc.scalar.activation(tmp[:], pw[:], AF.Exp, scale=0.125)
                        wex = wexp.tile([128, TOK], DT.bfloat16, tag="wx",
                                        bufs=6, name=f"wx{kt}_{hh}")
                        nc.vector.tensor_tensor(wex[:], tmp[:], masks[kt][:],
                                                op=ALU.mult)
                        nc.tensor.matmul(oacc[hh][:],
                                         v[kt][:, 65 * hh:65 * (hh + 1)],
                                         wex[:], start=(kt == 0), stop=(kt == 7))
                for hh in heads:
                    ct, sub = hh // 2, hh % 2
                    s_sb = rowp.tile([1, TOK], DT.float32, tag="s_sb", bufs=2,
                                     name="s_sb")
                    nc.scalar.activation(s_sb[:], oacc[hh][64:65, :], AF.Identity)
                    r_sb = rowp.tile([1, TOK], DT.float32, tag="r_sb", bufs=2,
                                     name="r_sb")
                    nc.vector.reciprocal(r_sb[:], s_sb[:])
                    rb = ps_w.tile([64, TOK], DT.float32, tag="w", name="rb")
                    nc.tensor.matmul(rb[:], ones64f[:], r_sb[:], start=True,
                                     stop=True)
                    orw = rowp.tile([64, TOK], DT.bfloat16, tag="orw", bufs=2,
                                    name="orw")
                    nc.scalar.activation(orw[:], oacc[hh][0:64, :], AF.Identity)
                    nc.vector.tensor_tensor(o2[ct][64 * sub:64 * sub + 64, :],
                                            orw[:], rb[:], op=ALU.mult)

            # ---- out-projection + residual (token-local, no collective)
            for et in range(ET):
                wp_t = wpp.tile([128, E], DT.bfloat16, tag="wps", name=f"wp{l}_{et}")
                nc.sync.dma_start(wp_t[:], inp[f"wps{l}"][et])
                pd = ps_mm.tile([128, TOK], DT.float32, tag="mm", name="pd")
                for ct in range(ET):
                    nc.tensor.matmul(pd[:], wp_t[:, ct * 128:(ct + 1) * 128],
                                     o2[ct][:], start=(ct == 0), stop=(ct == ET - 1))
                nc.vector.tensor_tensor(x[et][:], x[et][:], pd[:], op=ALU.add)

            # ---- LN2 -> h
            ln_own(f"l{l}b")

            # ---- FFN1: ff[ft] = relu(W1 h + b1)
            ff = [ffp.tile([128, TOK], DT.bfloat16, tag=f"ff{ft}", name=f"ff{ft}")
                  for ft in range(FT)]
            for ft in range(FT):
                w1_t = w1p.tile([128, E], DT.bfloat16, tag="w1s", name=f"w1{l}_{ft}")
                nc.sync.dma_start(w1_t[:], inp[f"w1s{l}"][ft])
                pf = ps_mm.tile([128, TOK], DT.float32, tag="mm", name="pf")
                for et in range(ET):
                    nc.tensor.matmul(pf[:], w1_t[:, et * 128:(et + 1) * 128],
                                     h[et][:], start=(et == 0), stop=(et == ET - 1))
                nc.scalar.activation(ff[ft][:], pf[:], AF.Relu,
                                     bias=bft[:, ft:ft + 1])

            # ---- FFN2 + bias + residual
            for et in range(ET):
                w2_t = w2p.tile([128, FFD], DT.bfloat16, tag="w2s",
                                name=f"w2{l}_{et}")
                nc.sync.dma_start(w2_t[:], inp[f"w2s{l}"][et])
                pd2 = ps_mm.tile([128, TOK], DT.float32, tag="mm", name="pd2")
                for ft in range(FT):
                    nc.tensor.matmul(pd2[:], w2_t[:, ft * 128:(ft + 1) * 128],
                                     ff[ft][:], start=(ft == 0), stop=False)
                nc.tensor.matmul(pd2[:], b2r[0:1, 128 * et:128 * (et + 1)],
                                 ones256[:], start=False, stop=True)
                nc.vector.tensor_tensor(x[et][:], x[et][:], pd2[:], op=ALU.add)

        # ---- final LN + 8-way AllGather of hf
        ln_own("f")
        aghin = dram.tile([E, TOK], DT.bfloat16, tag="aghin", name="aghin")
        aghout = dram.tile([NCORES * E, TOK], DT.bfloat16, tag="aghout",
                           name="aghout", addr_space="Shared")
        for et in range(ET):
            nc.gpsimd.dma_start(aghin[128 * et:128 * (et + 1), :], h[et][:])
        nc.gpsimd.collective_compute(
            "AllGather", ALU.bypass, replica_groups=GROUPS8,
            ins=[aghin.opt()], outs=[aghout.opt()])

        # ---- lm_head: all 2048 tokens x own vocab slice
        hf = [hfp.tile([128, 2 * T], DT.bfloat16, tag=f"hf{et}", name=f"hf{et}")
              for et in range(ET)]
        for et in range(ET):
            for b in range(NCORES):
                nc.scalar.dma_start(
                    hf[et][:, TOK * b:TOK * (b + 1)],
                    aghout[E * b + 128 * et:E * b + 128 * (et + 1), :])
        for vt in range(VSP // VCH):
            emb = [embp.tile([128, VCH], DT.bfloat16, tag=f"emb{et}",
                             name=f"emb{et}") for et in range(ET)]
            for et in range(ET):
                nc.sync.dma_start(
                    emb[et][:],
                    embT_d[128 * et:128 * (et + 1), VCH * vt:VCH * (vt + 1)])
            for tt in range(2 * T // 128):
                pl = ps_mm.tile([128, VCH], DT.float32, tag="mm", name="pl")
                for et in range(ET):
                    nc.tensor.matmul(pl[:], hf[et][:, 128 * tt:128 * (tt + 1)],
                                     emb[et][:], start=(et == 0), stop=(et == ET - 1))
                lsb = lsbp.tile([128, VCH], DT.float32, tag="lsb", name="lsb")
                if tt % 2 == 0:
                    nc.scalar.activation(lsb[:], pl[:], AF.Identity)
                else:
                    nc.vector.tensor_copy(lsb[:], pl[:])
                nc.sync.dma_start(
                    out_d[128 * tt:128 * (tt + 1), VCH * vt:VCH * (vt + 1)],
                    lsb[:])

    _split_sync_waits(nc)
    return nc


_NC = None


def _strips(a2):
    """[R, C] -> [C//128, 128, R] with strip[j][p, 128*i + c] = a2[128*i+p, 128*j+c]."""
    R, C = a2.shape
    return np.ascontiguousarray(
        a2.reshape(R // 128, 128, C // 128, 128).transpose(2, 1, 0, 3)
        .reshape(C // 128, 128, R))


def _host_prep(inputs):
    """Fold LN params into weights, build per-core input maps."""
    f32 = np.float32
    g = {}
    for k, v_ in inputs.items():
        a = np.asarray(v_)
        g[k] = a if a.dtype in (np.int64, np.int32) else np.asarray(a, f32)
    idx = np.asarray(inputs["idx"])
    s = f32(E) ** -0.5

    shared = []
    for l in range(L):
        g1, b1v = g["ln1_g"][l], g["ln1_b"][l]
        g2, b2v = g["ln2_g"][l], g["ln2_b"][l]
        wqT = (g["Wq"][l] * g1[None, :] * s).T.astype(f32)      # [E, ch]
        wkT = (g["Wk"][l] * g1[None, :] * s).T.astype(f32)
        wpT = (g["Wp"][l] * s).T.astype(f32)                    # [ch, E]
        wvT = np.zeros((E, VRT * 128), f32)                     # padded 1152
        bv = np.zeros((1, VRT * 128), f32)
        for hh in range(H):
            wsl = g["Wv"][l][hh * HD:(hh + 1) * HD]             # [64, E]
            wvT[:, hh * 65:hh * 65 + 64] = (wsl * g1[None, :] * s).T
            bv[0, hh * 65:hh * 65 + 64] = wsl @ b1v * s
            bv[0, hh * 65 + 64] = 1.0
        w1T = (g["W1"][l] * g2[None, :]).T.astype(f32)          # [E, F]
        w2T = np.ascontiguousarray(g["W2"][l].T)                # [F, E]
        d = {
            "wqs": _strips(wqT).astype(bf16),
            "bq": np.ascontiguousarray(
                (g["Wq"][l] @ b1v * s).reshape(ET, 128).T).astype(f32),
            "wks": _strips(wkT).astype(bf16),
            "bk": np.ascontiguousarray(
                (g["Wk"][l] @ b1v * s).reshape(ET, 128).T).astype(f32),
            "wvs": _strips(wvT).astype(bf16),
            "bvr": bv.astype(bf16),
            "wps": _strips(wpT).astype(bf16),
            "w1s": _strips(w1T).astype(bf16),
            "bf": np.ascontiguousarray(
                (g["W1"][l] @ b2v + g["b1"][l]).reshape(FT, 128).T).astype(f32),
            "w2s": _strips(w2T).astype(bf16),
            "b2r": g["b2"][l].reshape(1, E).astype(bf16),
        }
        shared.append(d)

    embTs, hbias = [], []
    for c in range(NCORES):
        vsl = slice(VS * c, VS * (c + 1))
        e = (g["tok_emb"][vsl] * g["lnf_g"][None, :]).T          # [E, 4000]
        ep = np.zeros((E, VSP), f32)
        ep[:, :VS] = e
        embTs.append(ep.astype(bf16))
        hbias.append(g["tok_emb"][vsl] @ g["lnf_b"] + g["head_b"][vsl])

    # masks per token-quarter r
    masks_r = []
    for r in range(SP):
        m = np.zeros((8, 128, TOK), f32)
        ii = np.arange(128)[:, None]
        jj = np.arange(TOK)[None, :]
        for kt in range(8):
            m[kt] = (128 * kt + ii <= TOK * r + jj).astype(f32)
        masks_r.append(m.astype(bf16))

    in_maps = []
    for c in range(NCORES):
        gb, r = c // SP, c % SP
        x0 = (g["tok_emb"][idx[gb, TOK * r:TOK * (r + 1)]]
              + g["pos_emb"][TOK * r:TOK * (r + 1)])             # [256, E]
        m = {
            "x0T": np.ascontiguousarray(x0.T).astype(bf16),
            "masks": masks_r[r],
            "invE": np.full((128, 1), 1.0 / E, bf16),
            "ones256": np.ones((1, TOK), bf16),
            "ones64f": np.ones((1, 64), f32),
            "embT": embTs[c],
        }
        for l in range(L):
            for k, v_ in shared[l].items():
                m[f"{k}{l}"] = v_
        in_maps.append(m)
    return in_maps, hbias


LAST_RESULT = None


def kernel(**inputs):
    global _NC, LAST_RESULT
    if _NC is None:
        _NC = _build_program()
    in_maps, hbias = _host_prep(inputs)
    import os
    trace = bool(os.environ.get("KBENCH_TRACE"))
    kw = {}
    if trace:
        import tempfile
        td = os.environ.get("KBENCH_TRACE_DIR")
        if td:
            os.makedirs(td, exist_ok=True)
        else:
            td = tempfile.mkdtemp(prefix="kbench_trace_")
        kw = dict(trace=True, tmpdir=td)
    res = run_bass_kernel_spmd(_NC, in_maps, list(range(NCORES)), **kw)
    LAST_RESULT = res
    B = 2
    logits = np.empty((B, T, V), np.float32)
    for c in range(NCORES):
        lt = res.results[c]["out"]                               # [2048, 4032]
        for j in range(NCORES):
            gb, r = j // SP, j % SP
            logits[gb, TOK * r:TOK * (r + 1), VS * c:VS * (c + 1)] = (
                lt[TOK * j:TOK * (j + 1), :VS] + hbias[c][None, :])
    return logits
